# revision 64
# baseline (speedup 1.0000x reference)
"""Trainium2 Bass kernel for nn_MultiHeadAttention (B=1, S=4096, D=2048, H=16, HD=128).

Sharding: tensor-parallel over heads — 2 heads per core on 8 NeuronCores.
Each core computes its 2 heads' Q/K/V projections, causal attention, and a
partial output projection (row-split Wo); the host sums the 8 partials and
adds the output bias (the all-reduce/unshard step).

Layout strategy (per core, all matmuls bf16 with fp32 PSUM accumulation):
  - X^T [2048, 4096] uploaded (e-major) so projections contract over e.
    DMA'd sb-major via ONE strided descriptor per 512-seq-col block (DMA
    trigger instructions cost ~0.6us each on the sync engine), so the first
    projection matmuls start after ~2 MB instead of after the full 16 MB.
  - Q, K produced transposed: QT/KT [d, s]. Scores computed transposed,
    S^T[k, q] = KT_tile^T @ QT, so p = exp(S^T) has k on partitions and
    attn@V needs no transpose. Q/K bias is fused into the PSUM drain on the
    (then-idle) ACT engine via activation(Identity, bias=per-partition AP).
  - psS is a 4-deep ring of single-bank [128,512] tiles, one exp per
    k-tile — deep enough that score matmuls never WAR-wait on exp.
  - Causal diagonal k-tiles stream only the valid q-columns (>= 128*jj into
    the q-block) through scores/exp/attn@V/denominator; the causal mask is a
    single shared [128,128] additive triangle applied via a 128-col
    identity-weight matmul.
  - Softmax denominators: a 3-level DVE adder tree (bf16, off the critical
    path) compresses eight k-tiles of p into one tile, so PE streams just
    ONE ones-column matmul per eight k-tiles into a single PSUM bank (h0
    row 0, h1 row 32 via tile_position).  Each removed matmul also removes
    a PE<->DVE coupling point, which pays ~2x its raw cycles. 1/denom via
    reciprocal_approx_fast (staged to SBUF first: the custom DVE op reads
    garbage from PSUM on hardware even though CoreSim accepts it), then
    partition-broadcast on the idle GpSimd engine (SBUF->SBUF). Each head's
    chain launches right after its last denominator; the psO scalings are
    deferred until both broadcasts are in flight so no DVE op parks waiting
    on GpSimd and blocks the in-order DVE queue.
  - O-projection (row-split Wo, accumulated over both local heads) drains
    from a global work queue at <=2 items per attention pair (outt regions
    are never overwritten, so items drift arbitrarily late), emitted right
    before each pair's scores where exp-gated PE stalls would land; not
    during qb=1, whose items would wait on the projection-phase DVE
    backlog. The tail q-block alternates psF/psD banks.
  - Output partials are staged per s-tile in [128, 2048] bf16 tiles (one
    output DMA per s-tile) and summed in fp32 on the host.

  - GpSimd warm-up: the engine loads its custom-op ucode library lazily on
    first use (~7us, observed as UNLOAD_LIB/LOAD_LIB in the profile) — a
    dummy partition_broadcast at kernel start hides the load under the X^T
    DMA instead of eating it at attention start.

Built with bacc.Bacc (event-semaphore chains for multi-wait sync).
Measured on TRN2: ~412-415 us/core HW exec (baseline 628 us), rel err
~6e-3 vs the fp32 reference.
"""

import numpy as np
import ml_dtypes

import concourse.bass as bass
import concourse.mybir as mybir
import concourse.tile as tile
from concourse import bacc
from concourse.bass_utils import run_bass_kernel_spmd


S = 4096          # sequence length
D = 2048          # model dim
NCORES = 8
DL = D // NCORES  # 256 local head dims (2 heads)
NH = 2            # heads per core
HD = 128          # head dim
QB = 512          # q block width
NQB = S // QB     # 8
KT = 128          # k tile (partitions)
NKT = S // KT     # 32
ET = 128          # e contraction tile
NET = D // ET     # 16
NST = S // 128    # 32 s-tiles
SCALE = 1.0 / np.sqrt(HD)

BF16 = mybir.dt.bfloat16
F32 = mybir.dt.float32


def build_nc(is_causal: bool) -> bass.Bass:
    nc = bacc.Bacc()

    XT = nc.dram_tensor("xt", [D, S], BF16, kind="ExternalInput")
    WQT = nc.dram_tensor("wqt", [D, DL], BF16, kind="ExternalInput")
    WKT = nc.dram_tensor("wkt", [D, DL], BF16, kind="ExternalInput")
    WVT = nc.dram_tensor("wvt", [D, DL], BF16, kind="ExternalInput")
    # bias columns [128, 4]: bq.d0 | bq.d1 | bk.d0 | bk.d1
    BQKC = nc.dram_tensor("bqkc", [128, 4], F32, kind="ExternalInput")
    BVROW = nc.dram_tensor("bvrow", [1, DL], BF16, kind="ExternalInput")
    WOT = nc.dram_tensor("wot", [DL, D], BF16, kind="ExternalInput")
    # masks[0]: additive causal triangle (0 / -1e9); masks[1]: identity
    MASKS = nc.dram_tensor("masks", [2, 128, 128], BF16, kind="ExternalInput")
    OUT = nc.dram_tensor("out", [S, D], BF16, kind="ExternalOutput")

    with tile.TileContext(nc) as tc:
        with tc.tile_pool(name="persist", bufs=1) as persist:
            # Q head0 | Q head1 | K head0 | K head1, each [128, 4096]
            qkt = persist.tile([128, 4 * S], BF16, name="qkt")
            # V natural layout: s-tile st at cols [st*256, (st+1)*256), head h at +h*128
            vt = persist.tile([128, NST * DL], BF16, name="vt")
            ones_col = persist.tile([128, 1], BF16, name="ones_col")
            ones_row = persist.tile([1, 128], BF16, name="ones_row")
            biasqk = persist.tile([128, 4], F32, name="biasqk")
            bvrow_sb = persist.tile([1, DL], BF16, name="bvrow_sb")
            bvb_sb = persist.tile([128, DL], BF16, name="bvb_sb")
            masks_sb = persist.tile([128, 2 * 128], BF16, name="masks_sb")

            gp_warm_src = persist.tile([1, 16], F32, name="gp_warm_src")
            gp_warm = persist.tile([128, 16], F32, name="gp_warm")

            nc.vector.memset(ones_col[:, :], 1.0)
            nc.vector.memset(ones_row[:, :], 1.0)
            # GpSimd loads its custom-op ucode library lazily on first use
            # (~7us) — warm it up here, hidden under the X^T DMA, so the
            # first real partition_broadcast (q-block 0's normalize, right at
            # attention start) doesn't eat the load
            nc.vector.memset(gp_warm_src[:, :], 1.0)
            nc.gpsimd.partition_broadcast(gp_warm[:, :], gp_warm_src[:, :])
            nc.sync.dma_start(out=bvrow_sb[:, :], in_=BVROW[:, :])
            nc.sync.dma_start(out=biasqk[:, :], in_=BQKC[:, :])
            if is_causal:
                # needed by q-block 0's diagonal matmuls — don't let it queue
                # behind the 16 MB X^T stream
                nc.sync.dma_start(
                    out=masks_sb.rearrange("p (j c) -> p j c", j=2),
                    in_=MASKS.rearrange("j p c -> p j c"),
                )

            # ---------------- Phase 2: QKV projections (sb-major) ----------
            with tc.tile_pool(name="xtp", bufs=1) as xtp, \
                 tc.tile_pool(name="wp", bufs=1) as wp, \
                 tc.tile_pool(name="ps2", bufs=3, space="PSUM") as ps2:
                xt_sb = xtp.tile([128, NET * S], BF16, name="xt_sb")
                wv_sb = wp.tile([128, NET * DL], BF16, name="wv_sb", tag="wv")
                wk_sb = wp.tile([128, NET * DL], BF16, name="wk_sb", tag="wk")
                wq_sb = wp.tile([128, NET * DL], BF16, name="wq_sb", tag="wq")
                # one strided DMA per transfer: DMA trigger instructions cost
                # ~0.6us each on the sync engine, so batching matters
                xt3 = xt_sb.rearrange("p (et s) -> p et s", et=NET)
                XT3 = XT.rearrange("(et p) s -> p et s", p=128)

                def dma_xt_block(sb):
                    nc.sync.dma_start(
                        out=xt3[:, :, sb * QB : (sb + 1) * QB],
                        in_=XT3[:, :, sb * QB : (sb + 1) * QB],
                    )

                wv3o = wv_sb.rearrange("p (et d) -> p et d", et=NET)
                wv3i = WVT.rearrange("(et p) d -> p et d", p=128)
                nc.sync.dma_start(out=wv3o[:, 0:8, :], in_=wv3i[:, 0:8, :])
                # first s-block in 256-col chunks (512B DMA lines — 128-col
                # chunks quarter the line size and throttle the cold DMA
                # engine) so the first V s-tiles' matmuls start early; wv-hi
                # right after the first chunk
                nc.sync.dma_start(
                    out=xt3[:, :, 0:256], in_=XT3[:, :, 0:256]
                )
                nc.sync.dma_start(out=wv3o[:, 8:16, :], in_=wv3i[:, 8:16, :])
                nc.sync.dma_start(
                    out=xt3[:, :, 256:512], in_=XT3[:, :, 256:512]
                )
                nc.sync.dma_start(
                    out=wk_sb.rearrange("p (et d) -> p et d", et=NET),
                    in_=WKT.rearrange("(et p) d -> p et d", p=128),
                )
                nc.sync.dma_start(
                    out=wq_sb.rearrange("p (et d) -> p et d", et=NET),
                    in_=WQT.rearrange("(et p) d -> p et d", p=128),
                )

                # broadcast bv across partitions once
                psb = ps2.tile([128, DL], F32, name="psb", tag="psv")
                nc.tensor.matmul(
                    psb[:, :], lhsT=ones_row[:, :], rhs=bvrow_sb[:, :],
                    start=True, stop=True,
                )
                nc.vector.tensor_copy(bvb_sb[:, :], psb[:, :])

                for sb in range(NQB):
                    if sb > 0:
                        dma_xt_block(sb)
                    # V for the 4 s-tiles of this block
                    for st4 in range(4):
                        st = 4 * sb + st4
                        psv = ps2.tile([128, DL], F32, name="psv", tag="psv")
                        for et in range(NET):
                            nc.tensor.matmul(
                                psv[:, :],
                                lhsT=xt_sb[:, et * S + st * 128 : et * S + (st + 1) * 128],
                                rhs=wv_sb[:, et * DL : (et + 1) * DL],
                                start=(et == 0),
                                stop=(et == NET - 1),
                            )
                        nc.vector.scalar_tensor_tensor(
                            out=vt[:, st * DL : (st + 1) * DL],
                            in0=psv[:, :],
                            scalar=1.0,
                            in1=bvb_sb[:, :],
                            op0=mybir.AluOpType.mult,
                            op1=mybir.AluOpType.add,
                        )
                    # K then Q for this block; bias fused into the ACT drain
                    for w_sb, base4, bias_base in (
                        (wk_sb, 2, 2), (wq_sb, 0, 0)
                    ):
                        for dt in range(NH):
                            psq = ps2.tile([128, QB], F32, name="psq", tag="psq")
                            for et in range(NET):
                                nc.tensor.matmul(
                                    psq[:, :],
                                    lhsT=w_sb[:, et * DL + dt * 128 : et * DL + (dt + 1) * 128],
                                    rhs=xt_sb[:, et * S + sb * QB : et * S + (sb + 1) * QB],
                                    start=(et == 0),
                                    stop=(et == NET - 1),
                                )
                            nc.scalar.activation(
                                qkt[:, (base4 + dt) * S + sb * QB : (base4 + dt) * S + (sb + 1) * QB],
                                psq[:, :],
                                mybir.ActivationFunctionType.Identity,
                                bias=biasqk[:, bias_base + dt : bias_base + dt + 1],
                                scale=1.0,
                            )

            # ------- Phases 3+4: attention with interleaved O-projection ---
            with tc.tile_pool(name="mid", bufs=1) as mid, \
                 tc.tile_pool(name="psO", bufs=2, space="PSUM") as psO_p, \
                 tc.tile_pool(name="psD", bufs=1, space="PSUM") as psD_p, \
                 tc.tile_pool(name="psS", bufs=4, space="PSUM") as psS_p, \
                 tc.tile_pool(name="psF", bufs=1, space="PSUM") as psF_p, \
                 tc.tile_pool(name="pp", bufs=10) as pp, \
                 tc.tile_pool(name="rp", bufs=2) as rp, \
                 tc.tile_pool(name="rbp", bufs=2) as rbp, \
                 tc.tile_pool(name="op", bufs=4) as op:
                # normalized attention outputs, transposed: (h*NQB+qb) tile [128d, 512q]
                outt = mid.tile([128, NH * NQB * QB], BF16, name="outt")
                wot_sb = mid.tile([128, NH * D], BF16, name="wot_sb")
                nc.sync.dma_start(
                    out=wot_sb.rearrange("p (h e) -> p h e", h=NH),
                    in_=WOT.rearrange("(h p) e -> p h e", p=128),
                )

                osb_open: dict = {}

                def emit_proj(qb0: int, j: int, et: int, alt: bool = False):
                    # O-projection for s-tile (qb0,j), e-chunk et; both heads
                    # accumulate in one psF bank, drained to bf16.  The four
                    # e-chunks of an s-tile share one osb staging tile so each
                    # s-tile costs a single output DMA.  In the tail (alt),
                    # items alternate into the psD bank (free after the last
                    # normalize) for a 2-deep psF rotation.
                    st = qb0 * 4 + j
                    if alt:
                        psF = psD_p.tile([128, 512], F32, name="psFt", tag="psD")
                    else:
                        psF = psF_p.tile([128, 512], F32, name="psF", tag="psF")
                    for h in range(NH):
                        o_base = (h * NQB + qb0) * QB + j * 128
                        nc.tensor.matmul(
                            psF[:, :],
                            lhsT=outt[:, o_base : o_base + 128],
                            rhs=wot_sb[:, h * D + et * 512 : h * D + (et + 1) * 512],
                            start=(h == 0),
                            stop=(h == NH - 1),
                        )
                    if st not in osb_open:
                        osb_open[st] = op.tile([128, D], BF16, name="osb", tag="osb")
                    osb = osb_open[st]
                    nc.vector.tensor_copy(
                        osb[:, et * 512 : (et + 1) * 512], psF[:, :]
                    )
                    if et == 3:
                        nc.sync.dma_start(
                            out=OUT[st * 128 : (st + 1) * 128, :],
                            in_=osb[:, :],
                        )
                        del osb_open[st]

                proj_items: list = []

                for qb in range(NQB):
                    n_k = 4 * (qb + 1) if is_causal else NKT
                    psO = {}
                    for h in range(NH):
                        psO[h] = psO_p.tile([128, QB], F32, name="psO", tag="psO")
                    psD = psD_p.tile([128, QB], F32, name="psD", tag="psD")
                    npairs = n_k // 2

                    norm_rb = {}
                    pend = {}
                    pend4 = {}
                    first_mm = {0: True, 1: True}

                    def emit_norm_pre(h, psD=psD):
                        # 1/denom chain for one head, launched right after its
                        # last denominator so it overlaps the other head's
                        # attn@V work.  The psO scaling is deferred (see
                        # emit_norm_mul) so no DVE op parks waiting on the
                        # GpSimd broadcast and blocks the in-order DVE queue.
                        dsb = rp.tile([1, QB], F32, name="dsb", tag="dsb", bufs=2)
                        nc.scalar.copy(dsb[:, :], psD[32 * h : 32 * h + 1, :])
                        recipf = rp.tile(
                            [1, QB], F32, name="recipf", tag="recipf", bufs=2
                        )
                        nc.vector.reciprocal_approx_fast(
                            out=recipf[:, :], in_=dsb[:, :]
                        )
                        rb = rbp.tile([128, QB], F32, name="rb", tag="rb")
                        nc.gpsimd.partition_broadcast(rb[:, :], recipf[:, :])
                        norm_rb[h] = rb

                    def emit_norm_mul(h, psO=psO, qb=qb):
                        o_base = (h * NQB + qb) * QB
                        nc.vector.tensor_mul(
                            outt[:, o_base : o_base + QB],
                            psO[h][:, :],
                            norm_rb[h][:, :],
                        )

                    for pi in range(npairs):
                        kt0 = 2 * pi
                        # scores + exp for both heads first (ACT gets a head
                        # start while the PE streams the other head's scores)
                        ps_info = {}
                        for h in range(NH):
                            diag = is_causal and (kt0 + 1 >= 4 * qb)
                            offs = (
                                (128 * (kt0 - 4 * qb), 128 * (kt0 + 1 - 4 * qb))
                                if diag
                                else (0, 0)
                            )
                            p = pp.tile([128, 2 * QB], BF16, name="p", tag="p")
                            # one psS bank + one exp per k-tile: a 4-deep psS
                            # ring decouples the PE from exp completion better
                            # than 2 two-bank megatiles
                            for u in range(2):
                                kt = kt0 + u
                                off = offs[u]
                                w = QB - off
                                psS = psS_p.tile([128, QB], F32, name="psS", tag="psS")
                                nc.tensor.matmul(
                                    psS[:, 0:w],
                                    lhsT=qkt[:, (2 + h) * S + kt * 128 : (2 + h) * S + (kt + 1) * 128],
                                    rhs=qkt[:, h * S + qb * QB + off : h * S + (qb + 1) * QB],
                                    start=True,
                                    stop=not diag,
                                )
                                if diag:
                                    # additive triangle on the first 128 valid
                                    # cols via identity-weight matmul
                                    nc.tensor.matmul(
                                        psS[:, 0:128],
                                        lhsT=masks_sb[:, 128:256],
                                        rhs=masks_sb[:, 0:128],
                                        start=False,
                                        stop=True,
                                    )
                                # exp lands q-aligned in p; pad cols are
                                # never read (attn@V / denom are narrowed)
                                nc.scalar.activation(
                                    p[:, u * QB + off : (u + 1) * QB],
                                    psS[:, 0:w],
                                    mybir.ActivationFunctionType.Exp,
                                    scale=float(SCALE),
                                )
                            ps_info[h] = (p, offs)
                        # attn@V + denominators, narrowed to the causally
                        # valid q-columns on diagonal k-tiles
                        for h in range(NH):
                            p, offs = ps_info[h]
                            for u in range(2):
                                kt = kt0 + u
                                off = offs[u]
                                nc.tensor.matmul(
                                    psO[h][:, off:QB],
                                    lhsT=vt[:, kt * DL + h * 128 : kt * DL + (h + 1) * 128],
                                    rhs=p[:, u * QB + off : (u + 1) * QB],
                                    start=(kt == 0),
                                    stop=(kt == n_k - 1),
                                )
                            # denominator: DVE pre-sums the two k-tiles (bf16,
                            # off the critical path), then ONE ones-matmul per
                            # pair — halves the PE denominator column count
                            off0, off1 = offs
                            psum2 = pp.tile([128, QB], BF16, name="psum2", tag="ps2", bufs=4)
                            if off1 > off0:
                                # u0's exclusive strip, then the common range
                                nc.vector.tensor_copy(
                                    psum2[:, off0:off1], p[:, off0:off1]
                                )
                                nc.vector.tensor_add(
                                    psum2[:, off1:QB],
                                    p[:, off1:QB],
                                    p[:, QB + off1 : 2 * QB],
                                )
                            else:
                                nc.vector.tensor_add(
                                    psum2[:, :], p[:, 0:QB], p[:, QB : 2 * QB]
                                )
                            # second tree level: combine two pair-sums on DVE
                            # so PE streams one ones-matmul per FOUR k-tiles
                            if pi % 2 == 0:
                                pend[h] = (psum2, off0)
                            else:
                                prev, poff = pend.pop(h)
                                psum4 = pp.tile(
                                    [128, QB], BF16, name="psum4", tag="ps4", bufs=3
                                )
                                if off0 > poff:
                                    nc.vector.tensor_copy(
                                        psum4[:, poff:off0], prev[:, poff:off0]
                                    )
                                    nc.vector.tensor_add(
                                        psum4[:, off0:QB],
                                        prev[:, off0:QB],
                                        psum2[:, off0:QB],
                                    )
                                else:
                                    nc.vector.tensor_add(
                                        psum4[:, :], prev[:, :], psum2[:, :]
                                    )

                                # third tree level: psum4 is always full width
                                # (poff==0 — the diagonal only narrows inside
                                # psum2), so pair them too; PE streams one
                                # ones-matmul per EIGHT k-tiles
                                def emit_den(rhs_t, start, stop, h=h):
                                    nc.tensor.matmul(
                                        psD[32 * h : 32 * h + 1, :],
                                        lhsT=ones_col[:, :],
                                        rhs=rhs_t[:, :],
                                        start=start,
                                        stop=stop,
                                        tile_position=(0, 32 * h),
                                    )

                                if h in pend4:
                                    prev4 = pend4.pop(h)
                                    psum8 = pp.tile(
                                        [128, QB], BF16, name="psum8", tag="ps8", bufs=2
                                    )
                                    nc.vector.tensor_add(
                                        psum8[:, :], prev4[:, :], psum4[:, :]
                                    )
                                    emit_den(
                                        psum8, first_mm[h], pi == npairs - 1
                                    )
                                    first_mm[h] = False
                                elif pi == npairs - 1:
                                    # odd number of psum4s — emit the last one
                                    emit_den(psum4, first_mm[h], True)
                                    first_mm[h] = False
                                else:
                                    pend4[h] = psum4
                            if kt0 + 2 >= n_k:
                                emit_norm_pre(h)
                        if kt0 + 2 >= n_k:
                            for h in range(NH):
                                emit_norm_mul(h)
                        # deferred O-projection right before the next pair's
                        # scores — ready matmuls sit exactly where the psS
                        # WAR (exp completion) stall would otherwise land.
                        # Not during qb=1: the first item would wait on the
                        # DVE backlog (projection-phase V drains) ahead of
                        # qb0's normalize.
                        if qb >= 2:
                            for _ in range(2):
                                if proj_items:
                                    emit_proj(*proj_items.pop(0))

                    # this block's O-projection items join the global queue;
                    # they drain at <=2 per pair over the REMAINING blocks
                    # (outt regions are never overwritten, so items can drift
                    # arbitrarily late)
                    proj_items.extend(
                        (qb, j, et) for j in range(4) for et in range(4)
                    )

                # tail: O-projection of the last q-block, alternating PSUM
                # banks so drains overlap the next pair of matmuls
                ti = 0
                while proj_items:
                    emit_proj(*proj_items.pop(0), alt=(ti % 2 == 1))
                    ti += 1
    nc.finalize()
    return nc


def _bf16(a: np.ndarray) -> np.ndarray:
    return np.ascontiguousarray(a.astype(ml_dtypes.bfloat16))


def make_in_maps(X, Wq, bq, Wk, bk, Wv, bv, Wo, is_causal: bool):
    x2d = np.asarray(X, dtype=np.float32).reshape(S, D)
    xt = _bf16(x2d.T)
    masks = np.zeros((2, 128, 128), dtype=ml_dtypes.bfloat16)
    if is_causal:
        ki = np.arange(128)[:, None]
        cj = np.arange(128)[None, :]
        masks[0] = np.where(ki <= cj, 0.0, -1e9).astype(ml_dtypes.bfloat16)
        masks[1] = np.eye(128, dtype=ml_dtypes.bfloat16)

    in_maps = []
    for c in range(NCORES):
        sl = slice(c * DL, (c + 1) * DL)
        in_maps.append(
            {
                "xt": xt,
                "wqt": _bf16(np.asarray(Wq)[sl, :].T),
                "wkt": _bf16(np.asarray(Wk)[sl, :].T),
                "wvt": _bf16(np.asarray(Wv)[sl, :].T),
                "bqkc": np.ascontiguousarray(
                    np.stack(
                        [
                            np.asarray(bq, dtype=np.float32)[sl][:128],
                            np.asarray(bq, dtype=np.float32)[sl][128:],
                            np.asarray(bk, dtype=np.float32)[sl][:128],
                            np.asarray(bk, dtype=np.float32)[sl][128:],
                        ],
                        axis=1,
                    )
                ),
                "bvrow": _bf16(np.asarray(bv)[None, sl]),
                "wot": _bf16(np.asarray(Wo)[:, sl].T),
                "masks": masks,
            }
        )
    return in_maps


_NC_CACHE: dict = {}


def _get_nc(is_causal: bool) -> bass.Bass:
    if is_causal not in _NC_CACHE:
        _NC_CACHE[is_causal] = build_nc(is_causal)
    return _NC_CACHE[is_causal]


def kernel(X, Wq, bq, Wk, bk, Wv, bv, Wo, bo, is_causal, **run_kwargs):
    causal = bool(int(np.asarray(is_causal)))
    nc = _get_nc(causal)
    in_maps = make_in_maps(X, Wq, bq, Wk, bk, Wv, bv, Wo, causal)
    res = run_bass_kernel_spmd(nc, in_maps, core_ids=list(range(NCORES)), **run_kwargs)
    out = np.asarray(bo, dtype=np.float32)[None, :].repeat(S, axis=0)
    for c in range(NCORES):
        out += np.asarray(res.results[c]["out"], dtype=np.float32)
    return out.reshape(1, S, D)


# revision 66
# speedup vs baseline: 1.0050x; 1.0050x over previous
"""Trainium2 Bass kernel for nn_MultiHeadAttention (B=1, S=4096, D=2048, H=16, HD=128).

Sharding: tensor-parallel over heads — 2 heads per core on 8 NeuronCores.
Each core computes its 2 heads' Q/K/V projections, causal attention, and a
partial output projection (row-split Wo); the host sums the 8 partials and
adds the output bias (the all-reduce/unshard step).

Layout strategy (per core, all matmuls bf16 with fp32 PSUM accumulation):
  - X^T [2048, 4096] uploaded (e-major) so projections contract over e.
    DMA'd sb-major via ONE strided descriptor per 512-seq-col block (DMA
    trigger instructions cost ~0.6us each on the sync engine), so the first
    projection matmuls start after ~2 MB instead of after the full 16 MB.
  - Q, K produced transposed: QT/KT [d, s]. Scores computed transposed,
    S^T[k, q] = KT_tile^T @ QT, so p = exp(S^T) has k on partitions and
    attn@V needs no transpose. Q/K bias is fused into the PSUM drain on the
    (then-idle) ACT engine via activation(Identity, bias=per-partition AP).
  - psS is a 4-deep ring of single-bank [128,512] tiles, one exp per
    k-tile — deep enough that score matmuls never WAR-wait on exp.
  - Causal diagonal k-tiles stream only the valid q-columns (>= 128*jj into
    the q-block) through scores/exp/attn@V/denominator; the causal mask is a
    single shared [128,128] additive triangle applied via a 128-col
    identity-weight matmul.
  - Softmax denominators: a 3-level DVE adder tree (bf16, off the critical
    path) compresses eight k-tiles of p into one tile, so PE streams just
    ONE ones-column matmul per eight k-tiles into a single PSUM bank (h0
    row 0, h1 row 32 via tile_position).  Each removed matmul also removes
    a PE<->DVE coupling point, which pays ~2x its raw cycles. 1/denom via
    reciprocal_approx_fast (staged to SBUF first: the custom DVE op reads
    garbage from PSUM on hardware even though CoreSim accepts it), then
    partition-broadcast on the idle GpSimd engine (SBUF->SBUF). Each head's
    chain launches right after its last denominator; the psO scalings are
    deferred until both broadcasts are in flight so no DVE op parks waiting
    on GpSimd and blocks the in-order DVE queue.
  - O-projection (row-split Wo, accumulated over both local heads) drains
    from a global work queue at <=2 items per attention pair (outt regions
    are never overwritten, so items drift arbitrarily late), emitted right
    before each pair's scores where exp-gated PE stalls would land; not
    during qb=1, whose items would wait on the projection-phase DVE
    backlog. The tail q-block alternates psF/psD banks.
  - Output partials are staged per s-tile in [128, 2048] bf16 tiles (one
    output DMA per s-tile) and summed in fp32 on the host.

  - GpSimd warm-up: the engine loads its custom-op ucode library lazily on
    first use (~7us, observed as UNLOAD_LIB/LOAD_LIB in the profile) — a
    dummy partition_broadcast at kernel start hides the load under the X^T
    DMA instead of eating it at attention start.

Built with bacc.Bacc (event-semaphore chains for multi-wait sync).
Measured on TRN2: ~412-415 us/core HW exec (baseline 628 us), rel err
~6e-3 vs the fp32 reference.
"""

import numpy as np
import ml_dtypes

import concourse.bass as bass
import concourse.mybir as mybir
import concourse.tile as tile
from concourse import bacc
from concourse.bass_utils import run_bass_kernel_spmd


S = 4096          # sequence length
D = 2048          # model dim
NCORES = 8
DL = D // NCORES  # 256 local head dims (2 heads)
NH = 2            # heads per core
HD = 128          # head dim
QB = 512          # q block width
NQB = S // QB     # 8
KT = 128          # k tile (partitions)
NKT = S // KT     # 32
ET = 128          # e contraction tile
NET = D // ET     # 16
NST = S // 128    # 32 s-tiles
SCALE = 1.0 / np.sqrt(HD)

BF16 = mybir.dt.bfloat16
F32 = mybir.dt.float32


def build_nc(is_causal: bool) -> bass.Bass:
    nc = bacc.Bacc()

    XT = nc.dram_tensor("xt", [D, S], BF16, kind="ExternalInput")
    WQT = nc.dram_tensor("wqt", [D, DL], BF16, kind="ExternalInput")
    WKT = nc.dram_tensor("wkt", [D, DL], BF16, kind="ExternalInput")
    WVT = nc.dram_tensor("wvt", [D, DL], BF16, kind="ExternalInput")
    # bias columns [128, 4]: bq.d0 | bq.d1 | bk.d0 | bk.d1
    BQKC = nc.dram_tensor("bqkc", [128, 4], F32, kind="ExternalInput")
    BVROW = nc.dram_tensor("bvrow", [1, DL], BF16, kind="ExternalInput")
    WOT = nc.dram_tensor("wot", [DL, D], BF16, kind="ExternalInput")
    # masks[0]: additive causal triangle (0 / -1e9); masks[1]: identity
    MASKS = nc.dram_tensor("masks", [2, 128, 128], BF16, kind="ExternalInput")
    OUT = nc.dram_tensor("out", [S, D], BF16, kind="ExternalOutput")

    with tile.TileContext(nc) as tc:
        with tc.tile_pool(name="persist", bufs=1) as persist:
            # Q head0 | Q head1 | K head0 | K head1, each [128, 4096]
            qkt = persist.tile([128, 4 * S], BF16, name="qkt")
            # V natural layout: s-tile st at cols [st*256, (st+1)*256), head h at +h*128
            vt = persist.tile([128, NST * DL], BF16, name="vt")
            ones_col = persist.tile([128, 1], BF16, name="ones_col")
            ones_row = persist.tile([1, 128], BF16, name="ones_row")
            biasqk = persist.tile([128, 4], F32, name="biasqk")
            bvrow_sb = persist.tile([1, DL], BF16, name="bvrow_sb")
            bvb_sb = persist.tile([128, DL], BF16, name="bvb_sb")
            masks_sb = persist.tile([128, 2 * 128], BF16, name="masks_sb")

            gp_warm_src = persist.tile([1, 16], F32, name="gp_warm_src")
            gp_warm = persist.tile([128, 16], F32, name="gp_warm")

            nc.vector.memset(ones_col[:, :], 1.0)
            nc.vector.memset(ones_row[:, :], 1.0)
            # GpSimd loads its custom-op ucode library lazily on first use
            # (~7us) — warm it up here, hidden under the X^T DMA, so the
            # first real partition_broadcast (q-block 0's normalize, right at
            # attention start) doesn't eat the load
            nc.vector.memset(gp_warm_src[:, :], 1.0)
            nc.gpsimd.partition_broadcast(gp_warm[:, :], gp_warm_src[:, :])
            nc.sync.dma_start(out=bvrow_sb[:, :], in_=BVROW[:, :])
            nc.sync.dma_start(out=biasqk[:, :], in_=BQKC[:, :])
            if is_causal:
                # needed by q-block 0's diagonal matmuls — don't let it queue
                # behind the 16 MB X^T stream
                nc.sync.dma_start(
                    out=masks_sb.rearrange("p (j c) -> p j c", j=2),
                    in_=MASKS.rearrange("j p c -> p j c"),
                )

            # ---------------- Phase 2: QKV projections (sb-major) ----------
            with tc.tile_pool(name="xtp", bufs=1) as xtp, \
                 tc.tile_pool(name="wp", bufs=1) as wp, \
                 tc.tile_pool(name="ps2", bufs=3, space="PSUM") as ps2:
                xt_sb = xtp.tile([128, NET * S], BF16, name="xt_sb")
                wv_sb = wp.tile([128, NET * DL], BF16, name="wv_sb", tag="wv")
                wk_sb = wp.tile([128, NET * DL], BF16, name="wk_sb", tag="wk")
                wq_sb = wp.tile([128, NET * DL], BF16, name="wq_sb", tag="wq")
                # one strided DMA per transfer: DMA trigger instructions cost
                # ~0.6us each on the sync engine, so batching matters
                xt3 = xt_sb.rearrange("p (et s) -> p et s", et=NET)
                XT3 = XT.rearrange("(et p) s -> p et s", p=128)

                def dma_xt_block(sb):
                    nc.sync.dma_start(
                        out=xt3[:, :, sb * QB : (sb + 1) * QB],
                        in_=XT3[:, :, sb * QB : (sb + 1) * QB],
                    )

                wv3o = wv_sb.rearrange("p (et d) -> p et d", et=NET)
                wv3i = WVT.rearrange("(et p) d -> p et d", p=128)
                nc.sync.dma_start(out=wv3o[:, 0:8, :], in_=wv3i[:, 0:8, :])
                # first s-block in 256-col chunks (512B DMA lines — 128-col
                # chunks quarter the line size and throttle the cold DMA
                # engine) so the first V s-tiles' matmuls start early; wv-hi
                # right after the first chunk
                # two queues share the cold-start first chunk
                nc.sync.dma_start(
                    out=xt3[:, 0:8, 0:256], in_=XT3[:, 0:8, 0:256]
                )
                nc.sync.dma_start(
                    out=xt3[:, 8:16, 0:256], in_=XT3[:, 8:16, 0:256]
                )
                nc.sync.dma_start(out=wv3o[:, 8:16, :], in_=wv3i[:, 8:16, :])
                nc.sync.dma_start(
                    out=xt3[:, :, 256:512], in_=XT3[:, :, 256:512]
                )
                nc.sync.dma_start(
                    out=wk_sb.rearrange("p (et d) -> p et d", et=NET),
                    in_=WKT.rearrange("(et p) d -> p et d", p=128),
                )
                nc.sync.dma_start(
                    out=wq_sb.rearrange("p (et d) -> p et d", et=NET),
                    in_=WQT.rearrange("(et p) d -> p et d", p=128),
                )

                # broadcast bv across partitions once
                psb = ps2.tile([128, DL], F32, name="psb", tag="psv")
                nc.tensor.matmul(
                    psb[:, :], lhsT=ones_row[:, :], rhs=bvrow_sb[:, :],
                    start=True, stop=True,
                )
                nc.vector.tensor_copy(bvb_sb[:, :], psb[:, :])

                for sb in range(NQB):
                    if sb > 0:
                        dma_xt_block(sb)
                    # V for the 4 s-tiles of this block
                    for st4 in range(4):
                        st = 4 * sb + st4
                        psv = ps2.tile([128, DL], F32, name="psv", tag="psv")
                        for et in range(NET):
                            nc.tensor.matmul(
                                psv[:, :],
                                lhsT=xt_sb[:, et * S + st * 128 : et * S + (st + 1) * 128],
                                rhs=wv_sb[:, et * DL : (et + 1) * DL],
                                start=(et == 0),
                                stop=(et == NET - 1),
                            )
                        nc.vector.scalar_tensor_tensor(
                            out=vt[:, st * DL : (st + 1) * DL],
                            in0=psv[:, :],
                            scalar=1.0,
                            in1=bvb_sb[:, :],
                            op0=mybir.AluOpType.mult,
                            op1=mybir.AluOpType.add,
                        )
                    # K then Q for this block; bias fused into the ACT drain
                    for w_sb, base4, bias_base in (
                        (wk_sb, 2, 2), (wq_sb, 0, 0)
                    ):
                        for dt in range(NH):
                            psq = ps2.tile([128, QB], F32, name="psq", tag="psq")
                            for et in range(NET):
                                nc.tensor.matmul(
                                    psq[:, :],
                                    lhsT=w_sb[:, et * DL + dt * 128 : et * DL + (dt + 1) * 128],
                                    rhs=xt_sb[:, et * S + sb * QB : et * S + (sb + 1) * QB],
                                    start=(et == 0),
                                    stop=(et == NET - 1),
                                )
                            nc.scalar.activation(
                                qkt[:, (base4 + dt) * S + sb * QB : (base4 + dt) * S + (sb + 1) * QB],
                                psq[:, :],
                                mybir.ActivationFunctionType.Identity,
                                bias=biasqk[:, bias_base + dt : bias_base + dt + 1],
                                scale=1.0,
                            )

            # ------- Phases 3+4: attention with interleaved O-projection ---
            with tc.tile_pool(name="mid", bufs=1) as mid, \
                 tc.tile_pool(name="psO", bufs=2, space="PSUM") as psO_p, \
                 tc.tile_pool(name="psD", bufs=1, space="PSUM") as psD_p, \
                 tc.tile_pool(name="psS", bufs=4, space="PSUM") as psS_p, \
                 tc.tile_pool(name="psF", bufs=1, space="PSUM") as psF_p, \
                 tc.tile_pool(name="pp", bufs=10) as pp, \
                 tc.tile_pool(name="rp", bufs=2) as rp, \
                 tc.tile_pool(name="rbp", bufs=2) as rbp, \
                 tc.tile_pool(name="op", bufs=4) as op:
                # normalized attention outputs, transposed: (h*NQB+qb) tile [128d, 512q]
                outt = mid.tile([128, NH * NQB * QB], BF16, name="outt")
                wot_sb = mid.tile([128, NH * D], BF16, name="wot_sb")
                nc.sync.dma_start(
                    out=wot_sb.rearrange("p (h e) -> p h e", h=NH),
                    in_=WOT.rearrange("(h p) e -> p h e", p=128),
                )

                osb_open: dict = {}

                def emit_proj(qb0: int, j: int, et: int, alt: bool = False):
                    # O-projection for s-tile (qb0,j), e-chunk et; both heads
                    # accumulate in one psF bank, drained to bf16.  The four
                    # e-chunks of an s-tile share one osb staging tile so each
                    # s-tile costs a single output DMA.  In the tail (alt),
                    # items alternate into the psD bank (free after the last
                    # normalize) for a 2-deep psF rotation.
                    st = qb0 * 4 + j
                    if alt:
                        psF = psD_p.tile([128, 512], F32, name="psFt", tag="psD")
                    else:
                        psF = psF_p.tile([128, 512], F32, name="psF", tag="psF")
                    for h in range(NH):
                        o_base = (h * NQB + qb0) * QB + j * 128
                        nc.tensor.matmul(
                            psF[:, :],
                            lhsT=outt[:, o_base : o_base + 128],
                            rhs=wot_sb[:, h * D + et * 512 : h * D + (et + 1) * 512],
                            start=(h == 0),
                            stop=(h == NH - 1),
                        )
                    if st not in osb_open:
                        osb_open[st] = op.tile([128, D], BF16, name="osb", tag="osb")
                    osb = osb_open[st]
                    nc.vector.tensor_copy(
                        osb[:, et * 512 : (et + 1) * 512], psF[:, :]
                    )
                    if et == 3:
                        nc.sync.dma_start(
                            out=OUT[st * 128 : (st + 1) * 128, :],
                            in_=osb[:, :],
                        )
                        del osb_open[st]

                proj_items: list = []

                for qb in range(NQB):
                    n_k = 4 * (qb + 1) if is_causal else NKT
                    psO = {}
                    for h in range(NH):
                        psO[h] = psO_p.tile([128, QB], F32, name="psO", tag="psO")
                    psD = psD_p.tile([128, QB], F32, name="psD", tag="psD")
                    npairs = n_k // 2

                    norm_rb = {}
                    pend = {}
                    pend4 = {}
                    first_mm = {0: True, 1: True}

                    def emit_norm_pre(h, psD=psD):
                        # 1/denom chain for one head, launched right after its
                        # last denominator so it overlaps the other head's
                        # attn@V work.  The psO scaling is deferred (see
                        # emit_norm_mul) so no DVE op parks waiting on the
                        # GpSimd broadcast and blocks the in-order DVE queue.
                        dsb = rp.tile([1, QB], F32, name="dsb", tag="dsb", bufs=2)
                        nc.scalar.copy(dsb[:, :], psD[32 * h : 32 * h + 1, :])
                        recipf = rp.tile(
                            [1, QB], F32, name="recipf", tag="recipf", bufs=2
                        )
                        nc.vector.reciprocal_approx_fast(
                            out=recipf[:, :], in_=dsb[:, :]
                        )
                        rb = rbp.tile([128, QB], F32, name="rb", tag="rb")
                        nc.gpsimd.partition_broadcast(rb[:, :], recipf[:, :])
                        norm_rb[h] = rb

                    def emit_norm_mul(h, psO=psO, qb=qb):
                        o_base = (h * NQB + qb) * QB
                        nc.vector.tensor_mul(
                            outt[:, o_base : o_base + QB],
                            psO[h][:, :],
                            norm_rb[h][:, :],
                        )

                    for pi in range(npairs):
                        kt0 = 2 * pi
                        # scores + exp for both heads first (ACT gets a head
                        # start while the PE streams the other head's scores)
                        ps_info = {}
                        for h in range(NH):
                            diag = is_causal and (kt0 + 1 >= 4 * qb)
                            offs = (
                                (128 * (kt0 - 4 * qb), 128 * (kt0 + 1 - 4 * qb))
                                if diag
                                else (0, 0)
                            )
                            p = pp.tile([128, 2 * QB], BF16, name="p", tag="p")
                            # one psS bank + one exp per k-tile: a 4-deep psS
                            # ring decouples the PE from exp completion better
                            # than 2 two-bank megatiles
                            for u in range(2):
                                kt = kt0 + u
                                off = offs[u]
                                w = QB - off
                                psS = psS_p.tile([128, QB], F32, name="psS", tag="psS")
                                nc.tensor.matmul(
                                    psS[:, 0:w],
                                    lhsT=qkt[:, (2 + h) * S + kt * 128 : (2 + h) * S + (kt + 1) * 128],
                                    rhs=qkt[:, h * S + qb * QB + off : h * S + (qb + 1) * QB],
                                    start=True,
                                    stop=not diag,
                                )
                                if diag:
                                    # additive triangle on the first 128 valid
                                    # cols via identity-weight matmul
                                    nc.tensor.matmul(
                                        psS[:, 0:128],
                                        lhsT=masks_sb[:, 128:256],
                                        rhs=masks_sb[:, 0:128],
                                        start=False,
                                        stop=True,
                                    )
                                # exp lands q-aligned in p; pad cols are
                                # never read (attn@V / denom are narrowed)
                                nc.scalar.activation(
                                    p[:, u * QB + off : (u + 1) * QB],
                                    psS[:, 0:w],
                                    mybir.ActivationFunctionType.Exp,
                                    scale=float(SCALE),
                                )
                            ps_info[h] = (p, offs)
                        # attn@V + denominators, narrowed to the causally
                        # valid q-columns on diagonal k-tiles
                        for h in range(NH):
                            p, offs = ps_info[h]
                            for u in range(2):
                                kt = kt0 + u
                                off = offs[u]
                                nc.tensor.matmul(
                                    psO[h][:, off:QB],
                                    lhsT=vt[:, kt * DL + h * 128 : kt * DL + (h + 1) * 128],
                                    rhs=p[:, u * QB + off : (u + 1) * QB],
                                    start=(kt == 0),
                                    stop=(kt == n_k - 1),
                                )
                            # denominator: DVE pre-sums the two k-tiles (bf16,
                            # off the critical path), then ONE ones-matmul per
                            # pair — halves the PE denominator column count
                            off0, off1 = offs
                            psum2 = pp.tile([128, QB], BF16, name="psum2", tag="ps2", bufs=4)
                            if off1 > off0:
                                # u0's exclusive strip, then the common range
                                nc.vector.tensor_copy(
                                    psum2[:, off0:off1], p[:, off0:off1]
                                )
                                nc.vector.tensor_add(
                                    psum2[:, off1:QB],
                                    p[:, off1:QB],
                                    p[:, QB + off1 : 2 * QB],
                                )
                            else:
                                nc.vector.tensor_add(
                                    psum2[:, :], p[:, 0:QB], p[:, QB : 2 * QB]
                                )
                            # second tree level: combine two pair-sums on DVE
                            # so PE streams one ones-matmul per FOUR k-tiles
                            if pi % 2 == 0:
                                pend[h] = (psum2, off0)
                            else:
                                prev, poff = pend.pop(h)
                                psum4 = pp.tile(
                                    [128, QB], BF16, name="psum4", tag="ps4", bufs=3
                                )
                                if off0 > poff:
                                    nc.vector.tensor_copy(
                                        psum4[:, poff:off0], prev[:, poff:off0]
                                    )
                                    nc.vector.tensor_add(
                                        psum4[:, off0:QB],
                                        prev[:, off0:QB],
                                        psum2[:, off0:QB],
                                    )
                                else:
                                    nc.vector.tensor_add(
                                        psum4[:, :], prev[:, :], psum2[:, :]
                                    )

                                # third tree level: psum4 is always full width
                                # (poff==0 — the diagonal only narrows inside
                                # psum2), so pair them too; PE streams one
                                # ones-matmul per EIGHT k-tiles
                                def emit_den(rhs_t, start, stop, h=h):
                                    nc.tensor.matmul(
                                        psD[32 * h : 32 * h + 1, :],
                                        lhsT=ones_col[:, :],
                                        rhs=rhs_t[:, :],
                                        start=start,
                                        stop=stop,
                                        tile_position=(0, 32 * h),
                                    )

                                if h in pend4:
                                    prev4 = pend4.pop(h)
                                    psum8 = pp.tile(
                                        [128, QB], BF16, name="psum8", tag="ps8", bufs=2
                                    )
                                    nc.vector.tensor_add(
                                        psum8[:, :], prev4[:, :], psum4[:, :]
                                    )
                                    emit_den(
                                        psum8, first_mm[h], pi == npairs - 1
                                    )
                                    first_mm[h] = False
                                elif pi == npairs - 1:
                                    # odd number of psum4s — emit the last one
                                    emit_den(psum4, first_mm[h], True)
                                    first_mm[h] = False
                                else:
                                    pend4[h] = psum4
                            if kt0 + 2 >= n_k:
                                emit_norm_pre(h)
                        if kt0 + 2 >= n_k:
                            for h in range(NH):
                                emit_norm_mul(h)
                        # deferred O-projection right before the next pair's
                        # scores — ready matmuls sit exactly where the psS
                        # WAR (exp completion) stall would otherwise land.
                        # Not during qb=1: the first item would wait on the
                        # DVE backlog (projection-phase V drains) ahead of
                        # qb0's normalize.
                        if qb >= 2:
                            # deeper pops early: qb2-3 still have exp-WAR
                            # stalls and a deep backlog of ready items
                            for _ in range(3 if qb <= 3 else 2):
                                if proj_items:
                                    emit_proj(*proj_items.pop(0))

                    # this block's O-projection items join the global queue;
                    # they drain at <=2 per pair over the REMAINING blocks
                    # (outt regions are never overwritten, so items can drift
                    # arbitrarily late)
                    proj_items.extend(
                        (qb, j, et) for j in range(4) for et in range(4)
                    )

                # tail: O-projection of the last q-block, alternating PSUM
                # banks so drains overlap the next pair of matmuls
                ti = 0
                while proj_items:
                    emit_proj(*proj_items.pop(0), alt=(ti % 2 == 1))
                    ti += 1
    nc.finalize()
    return nc


def _bf16(a: np.ndarray) -> np.ndarray:
    return np.ascontiguousarray(a.astype(ml_dtypes.bfloat16))


def make_in_maps(X, Wq, bq, Wk, bk, Wv, bv, Wo, is_causal: bool):
    x2d = np.asarray(X, dtype=np.float32).reshape(S, D)
    xt = _bf16(x2d.T)
    masks = np.zeros((2, 128, 128), dtype=ml_dtypes.bfloat16)
    if is_causal:
        ki = np.arange(128)[:, None]
        cj = np.arange(128)[None, :]
        masks[0] = np.where(ki <= cj, 0.0, -1e9).astype(ml_dtypes.bfloat16)
        masks[1] = np.eye(128, dtype=ml_dtypes.bfloat16)

    in_maps = []
    for c in range(NCORES):
        sl = slice(c * DL, (c + 1) * DL)
        in_maps.append(
            {
                "xt": xt,
                "wqt": _bf16(np.asarray(Wq)[sl, :].T),
                "wkt": _bf16(np.asarray(Wk)[sl, :].T),
                "wvt": _bf16(np.asarray(Wv)[sl, :].T),
                "bqkc": np.ascontiguousarray(
                    np.stack(
                        [
                            np.asarray(bq, dtype=np.float32)[sl][:128],
                            np.asarray(bq, dtype=np.float32)[sl][128:],
                            np.asarray(bk, dtype=np.float32)[sl][:128],
                            np.asarray(bk, dtype=np.float32)[sl][128:],
                        ],
                        axis=1,
                    )
                ),
                "bvrow": _bf16(np.asarray(bv)[None, sl]),
                "wot": _bf16(np.asarray(Wo)[:, sl].T),
                "masks": masks,
            }
        )
    return in_maps


_NC_CACHE: dict = {}


def _get_nc(is_causal: bool) -> bass.Bass:
    if is_causal not in _NC_CACHE:
        _NC_CACHE[is_causal] = build_nc(is_causal)
    return _NC_CACHE[is_causal]


def kernel(X, Wq, bq, Wk, bk, Wv, bv, Wo, bo, is_causal, **run_kwargs):
    causal = bool(int(np.asarray(is_causal)))
    nc = _get_nc(causal)
    in_maps = make_in_maps(X, Wq, bq, Wk, bk, Wv, bv, Wo, causal)
    res = run_bass_kernel_spmd(nc, in_maps, core_ids=list(range(NCORES)), **run_kwargs)
    out = np.asarray(bo, dtype=np.float32)[None, :].repeat(S, axis=0)
    for c in range(NCORES):
        out += np.asarray(res.results[c]["out"], dtype=np.float32)
    return out.reshape(1, S, D)


# revision 69
# speedup vs baseline: 1.0056x; 1.0006x over previous
"""Trainium2 Bass kernel for nn_MultiHeadAttention (B=1, S=4096, D=2048, H=16, HD=128).

Sharding: tensor-parallel over heads — 2 heads per core on 8 NeuronCores.
Each core computes its 2 heads' Q/K/V projections, causal attention, and a
partial output projection (row-split Wo); the host sums the 8 partials and
adds the output bias (the all-reduce/unshard step).

Layout strategy (per core, all matmuls bf16 with fp32 PSUM accumulation):
  - X^T [2048, 4096] uploaded (e-major) so projections contract over e.
    DMA'd sb-major via ONE strided descriptor per 512-seq-col block (DMA
    trigger instructions cost ~0.6us each on the sync engine), so the first
    projection matmuls start after ~2 MB instead of after the full 16 MB.
  - Q, K produced transposed: QT/KT [d, s]. Scores computed transposed,
    S^T[k, q] = KT_tile^T @ QT, so p = exp(S^T) has k on partitions and
    attn@V needs no transpose. Q/K bias is fused into the PSUM drain on the
    (then-idle) ACT engine via activation(Identity, bias=per-partition AP).
  - psS is a 4-deep ring of single-bank [128,512] tiles, one exp per
    k-tile — deep enough that score matmuls never WAR-wait on exp.
  - Causal diagonal k-tiles stream only the valid q-columns (>= 128*jj into
    the q-block) through scores/exp/attn@V/denominator; the causal mask is a
    single shared [128,128] additive triangle applied via a 128-col
    identity-weight matmul.
  - Softmax denominators: a 3-level DVE adder tree (bf16, off the critical
    path) compresses eight k-tiles of p into one tile, so PE streams just
    ONE ones-column matmul per eight k-tiles into a single PSUM bank (h0
    row 0, h1 row 32 via tile_position).  Each removed matmul also removes
    a PE<->DVE coupling point, which pays ~2x its raw cycles. 1/denom via
    reciprocal_approx_fast (staged to SBUF first: the custom DVE op reads
    garbage from PSUM on hardware even though CoreSim accepts it), then
    partition-broadcast on the idle GpSimd engine (SBUF->SBUF). Each head's
    chain launches right after its last denominator; the psO scalings are
    deferred until both broadcasts are in flight so no DVE op parks waiting
    on GpSimd and blocks the in-order DVE queue.
  - O-projection (row-split Wo, accumulated over both local heads) drains
    from a global work queue at <=2 items per attention pair (outt regions
    are never overwritten, so items drift arbitrarily late), emitted right
    before each pair's scores where exp-gated PE stalls would land; not
    during qb=1, whose items would wait on the projection-phase DVE
    backlog. The tail q-block alternates psF/psD banks.
  - Output partials are staged per s-tile in [128, 2048] bf16 tiles (one
    output DMA per s-tile) and summed in fp32 on the host.

  - GpSimd warm-up: the engine loads its custom-op ucode library lazily on
    first use (~7us, observed as UNLOAD_LIB/LOAD_LIB in the profile) — a
    dummy partition_broadcast at kernel start hides the load under the X^T
    DMA instead of eating it at attention start.

Built with bacc.Bacc (event-semaphore chains for multi-wait sync).
Measured on TRN2: ~412-415 us/core HW exec (baseline 628 us), rel err
~6e-3 vs the fp32 reference.
"""

import numpy as np
import ml_dtypes

import concourse.bass as bass
import concourse.mybir as mybir
import concourse.tile as tile
from concourse import bacc
from concourse.bass_utils import run_bass_kernel_spmd


S = 4096          # sequence length
D = 2048          # model dim
NCORES = 8
DL = D // NCORES  # 256 local head dims (2 heads)
NH = 2            # heads per core
HD = 128          # head dim
QB = 512          # q block width
NQB = S // QB     # 8
KT = 128          # k tile (partitions)
NKT = S // KT     # 32
ET = 128          # e contraction tile
NET = D // ET     # 16
NST = S // 128    # 32 s-tiles
SCALE = 1.0 / np.sqrt(HD)

BF16 = mybir.dt.bfloat16
F32 = mybir.dt.float32


def build_nc(is_causal: bool) -> bass.Bass:
    nc = bacc.Bacc()

    XT = nc.dram_tensor("xt", [D, S], BF16, kind="ExternalInput")
    WQT = nc.dram_tensor("wqt", [D, DL], BF16, kind="ExternalInput")
    WKT = nc.dram_tensor("wkt", [D, DL], BF16, kind="ExternalInput")
    WVT = nc.dram_tensor("wvt", [D, DL], BF16, kind="ExternalInput")
    # bias columns [128, 4]: bq.d0 | bq.d1 | bk.d0 | bk.d1
    BQKC = nc.dram_tensor("bqkc", [128, 4], F32, kind="ExternalInput")
    BVROW = nc.dram_tensor("bvrow", [1, DL], BF16, kind="ExternalInput")
    WOT = nc.dram_tensor("wot", [DL, D], BF16, kind="ExternalInput")
    # masks[0]: additive causal triangle (0 / -1e9); masks[1]: identity
    MASKS = nc.dram_tensor("masks", [2, 128, 128], BF16, kind="ExternalInput")
    OUT = nc.dram_tensor("out", [S, D], BF16, kind="ExternalOutput")

    with tile.TileContext(nc) as tc:
        with tc.tile_pool(name="persist", bufs=1) as persist:
            # Q head0 | Q head1 | K head0 | K head1, each [128, 4096]
            qkt = persist.tile([128, 4 * S], BF16, name="qkt")
            # V natural layout: s-tile st at cols [st*256, (st+1)*256), head h at +h*128
            vt = persist.tile([128, NST * DL], BF16, name="vt")
            ones_col = persist.tile([128, 1], BF16, name="ones_col")
            ones_row = persist.tile([1, 128], BF16, name="ones_row")
            biasqk = persist.tile([128, 4], F32, name="biasqk")
            bvrow_sb = persist.tile([1, DL], BF16, name="bvrow_sb")
            bvb_sb = persist.tile([128, DL], BF16, name="bvb_sb")
            masks_sb = persist.tile([128, 2 * 128], BF16, name="masks_sb")

            gp_warm_src = persist.tile([1, 16], F32, name="gp_warm_src")
            gp_warm = persist.tile([128, 16], F32, name="gp_warm")

            nc.vector.memset(ones_col[:, :], 1.0)
            nc.vector.memset(ones_row[:, :], 1.0)
            # GpSimd loads its custom-op ucode library lazily on first use
            # (~7us) — warm it up here, hidden under the X^T DMA, so the
            # first real partition_broadcast (q-block 0's normalize, right at
            # attention start) doesn't eat the load
            nc.vector.memset(gp_warm_src[:, :], 1.0)
            nc.gpsimd.partition_broadcast(gp_warm[:, :], gp_warm_src[:, :])
            nc.sync.dma_start(out=bvrow_sb[:, :], in_=BVROW[:, :])
            nc.sync.dma_start(out=biasqk[:, :], in_=BQKC[:, :])
            if is_causal:
                # needed by q-block 0's diagonal matmuls — don't let it queue
                # behind the 16 MB X^T stream
                nc.sync.dma_start(
                    out=masks_sb.rearrange("p (j c) -> p j c", j=2),
                    in_=MASKS.rearrange("j p c -> p j c"),
                )

            # ---------------- Phase 2: QKV projections (sb-major) ----------
            with tc.tile_pool(name="xtp", bufs=1) as xtp, \
                 tc.tile_pool(name="wp", bufs=1) as wp, \
                 tc.tile_pool(name="ps2", bufs=3, space="PSUM") as ps2:
                xt_sb = xtp.tile([128, NET * S], BF16, name="xt_sb")
                wv_sb = wp.tile([128, NET * DL], BF16, name="wv_sb", tag="wv")
                wk_sb = wp.tile([128, NET * DL], BF16, name="wk_sb", tag="wk")
                wq_sb = wp.tile([128, NET * DL], BF16, name="wq_sb", tag="wq")
                # one strided DMA per transfer: DMA trigger instructions cost
                # ~0.6us each on the sync engine, so batching matters
                xt3 = xt_sb.rearrange("p (et s) -> p et s", et=NET)
                XT3 = XT.rearrange("(et p) s -> p et s", p=128)

                def dma_xt_block(sb):
                    nc.sync.dma_start(
                        out=xt3[:, :, sb * QB : (sb + 1) * QB],
                        in_=XT3[:, :, sb * QB : (sb + 1) * QB],
                    )

                wv3o = wv_sb.rearrange("p (et d) -> p et d", et=NET)
                wv3i = WVT.rearrange("(et p) d -> p et d", p=128)
                nc.sync.dma_start(out=wv3o[:, 0:8, :], in_=wv3i[:, 0:8, :])
                # first s-block in 256-col chunks (512B DMA lines — 128-col
                # chunks quarter the line size and throttle the cold DMA
                # engine) so the first V s-tiles' matmuls start early; wv-hi
                # right after the first chunk
                # two queues share the cold-start first chunk
                nc.sync.dma_start(
                    out=xt3[:, 0:8, 0:256], in_=XT3[:, 0:8, 0:256]
                )
                nc.sync.dma_start(
                    out=xt3[:, 8:16, 0:256], in_=XT3[:, 8:16, 0:256]
                )
                nc.sync.dma_start(out=wv3o[:, 8:16, :], in_=wv3i[:, 8:16, :])
                nc.sync.dma_start(
                    out=xt3[:, :, 256:512], in_=XT3[:, :, 256:512]
                )
                nc.sync.dma_start(
                    out=wk_sb.rearrange("p (et d) -> p et d", et=NET),
                    in_=WKT.rearrange("(et p) d -> p et d", p=128),
                )
                nc.sync.dma_start(
                    out=wq_sb.rearrange("p (et d) -> p et d", et=NET),
                    in_=WQT.rearrange("(et p) d -> p et d", p=128),
                )

                # broadcast bv across partitions once
                psb = ps2.tile([128, DL], F32, name="psb", tag="psv")
                nc.tensor.matmul(
                    psb[:, :], lhsT=ones_row[:, :], rhs=bvrow_sb[:, :],
                    start=True, stop=True,
                )
                nc.vector.tensor_copy(bvb_sb[:, :], psb[:, :])

                for sb in range(NQB):
                    if sb > 0:
                        dma_xt_block(sb)
                    # V for the 4 s-tiles of this block
                    for st4 in range(4):
                        st = 4 * sb + st4
                        psv = ps2.tile([128, DL], F32, name="psv", tag="psv")
                        for et in range(NET):
                            nc.tensor.matmul(
                                psv[:, :],
                                lhsT=xt_sb[:, et * S + st * 128 : et * S + (st + 1) * 128],
                                rhs=wv_sb[:, et * DL : (et + 1) * DL],
                                start=(et == 0),
                                stop=(et == NET - 1),
                            )
                        nc.vector.scalar_tensor_tensor(
                            out=vt[:, st * DL : (st + 1) * DL],
                            in0=psv[:, :],
                            scalar=1.0,
                            in1=bvb_sb[:, :],
                            op0=mybir.AluOpType.mult,
                            op1=mybir.AluOpType.add,
                        )
                    # K then Q for this block; bias fused into the ACT drain
                    for w_sb, base4, bias_base in (
                        (wk_sb, 2, 2), (wq_sb, 0, 0)
                    ):
                        for dt in range(NH):
                            psq = ps2.tile([128, QB], F32, name="psq", tag="psq")
                            for et in range(NET):
                                nc.tensor.matmul(
                                    psq[:, :],
                                    lhsT=w_sb[:, et * DL + dt * 128 : et * DL + (dt + 1) * 128],
                                    rhs=xt_sb[:, et * S + sb * QB : et * S + (sb + 1) * QB],
                                    start=(et == 0),
                                    stop=(et == NET - 1),
                                )
                            nc.scalar.activation(
                                qkt[:, (base4 + dt) * S + sb * QB : (base4 + dt) * S + (sb + 1) * QB],
                                psq[:, :],
                                mybir.ActivationFunctionType.Identity,
                                bias=biasqk[:, bias_base + dt : bias_base + dt + 1],
                                scale=1.0,
                            )

            # ------- Phases 3+4: attention with interleaved O-projection ---
            with tc.tile_pool(name="mid", bufs=1) as mid, \
                 tc.tile_pool(name="psO", bufs=2, space="PSUM") as psO_p, \
                 tc.tile_pool(name="psD", bufs=1, space="PSUM") as psD_p, \
                 tc.tile_pool(name="psS", bufs=4, space="PSUM") as psS_p, \
                 tc.tile_pool(name="psF", bufs=1, space="PSUM") as psF_p, \
                 tc.tile_pool(name="pp", bufs=10) as pp, \
                 tc.tile_pool(name="rp", bufs=2) as rp, \
                 tc.tile_pool(name="rbp", bufs=2) as rbp, \
                 tc.tile_pool(name="op", bufs=4) as op:
                # normalized attention outputs, transposed: (h*NQB+qb) tile [128d, 512q]
                outt = mid.tile([128, NH * NQB * QB], BF16, name="outt")
                wot_sb = mid.tile([128, NH * D], BF16, name="wot_sb")
                nc.sync.dma_start(
                    out=wot_sb.rearrange("p (h e) -> p h e", h=NH),
                    in_=WOT.rearrange("(h p) e -> p h e", p=128),
                )

                osb_open: dict = {}

                def emit_proj(qb0: int, j: int, et: int, alt: bool = False):
                    # O-projection for s-tile (qb0,j), e-chunk et; both heads
                    # accumulate in one psF bank, drained to bf16.  The four
                    # e-chunks of an s-tile share one osb staging tile so each
                    # s-tile costs a single output DMA.  In the tail (alt),
                    # items alternate into the psD bank (free after the last
                    # normalize) for a 2-deep psF rotation.
                    st = qb0 * 4 + j
                    if alt:
                        psF = psD_p.tile([128, 512], F32, name="psFt", tag="psD")
                    else:
                        psF = psF_p.tile([128, 512], F32, name="psF", tag="psF")
                    for h in range(NH):
                        o_base = (h * NQB + qb0) * QB + j * 128
                        nc.tensor.matmul(
                            psF[:, :],
                            lhsT=outt[:, o_base : o_base + 128],
                            rhs=wot_sb[:, h * D + et * 512 : h * D + (et + 1) * 512],
                            start=(h == 0),
                            stop=(h == NH - 1),
                        )
                    if st not in osb_open:
                        osb_open[st] = op.tile([128, D], BF16, name="osb", tag="osb")
                    osb = osb_open[st]
                    nc.vector.tensor_copy(
                        osb[:, et * 512 : (et + 1) * 512], psF[:, :]
                    )
                    if et == 3:
                        nc.sync.dma_start(
                            out=OUT[st * 128 : (st + 1) * 128, :],
                            in_=osb[:, :],
                        )
                        del osb_open[st]

                proj_items: list = []

                for qb in range(NQB):
                    n_k = 4 * (qb + 1) if is_causal else NKT
                    psO = {}
                    for h in range(NH):
                        psO[h] = psO_p.tile([128, QB], F32, name="psO", tag="psO")
                    psD = psD_p.tile([128, QB], F32, name="psD", tag="psD")
                    npairs = n_k // 2

                    norm_rb = {}
                    pend = {}
                    pend4 = {}
                    pend8 = {}
                    first_mm = {0: True, 1: True}

                    def emit_norm_pre(h, psD=psD):
                        # 1/denom chain for one head, launched right after its
                        # last denominator so it overlaps the other head's
                        # attn@V work.  The psO scaling is deferred (see
                        # emit_norm_mul) so no DVE op parks waiting on the
                        # GpSimd broadcast and blocks the in-order DVE queue.
                        dsb = rp.tile([1, QB], F32, name="dsb", tag="dsb", bufs=2)
                        nc.scalar.copy(dsb[:, :], psD[32 * h : 32 * h + 1, :])
                        recipf = rp.tile(
                            [1, QB], F32, name="recipf", tag="recipf", bufs=2
                        )
                        nc.vector.reciprocal_approx_fast(
                            out=recipf[:, :], in_=dsb[:, :]
                        )
                        rb = rbp.tile([128, QB], F32, name="rb", tag="rb")
                        nc.gpsimd.partition_broadcast(rb[:, :], recipf[:, :])
                        norm_rb[h] = rb

                    def emit_norm_mul(h, psO=psO, qb=qb):
                        o_base = (h * NQB + qb) * QB
                        nc.vector.tensor_mul(
                            outt[:, o_base : o_base + QB],
                            psO[h][:, :],
                            norm_rb[h][:, :],
                        )

                    for pi in range(npairs):
                        kt0 = 2 * pi
                        # scores + exp for both heads first (ACT gets a head
                        # start while the PE streams the other head's scores)
                        ps_info = {}
                        for h in range(NH):
                            diag = is_causal and (kt0 + 1 >= 4 * qb)
                            offs = (
                                (128 * (kt0 - 4 * qb), 128 * (kt0 + 1 - 4 * qb))
                                if diag
                                else (0, 0)
                            )
                            p = pp.tile([128, 2 * QB], BF16, name="p", tag="p")
                            # one psS bank + one exp per k-tile: a 4-deep psS
                            # ring decouples the PE from exp completion better
                            # than 2 two-bank megatiles
                            for u in range(2):
                                kt = kt0 + u
                                off = offs[u]
                                w = QB - off
                                psS = psS_p.tile([128, QB], F32, name="psS", tag="psS")
                                nc.tensor.matmul(
                                    psS[:, 0:w],
                                    lhsT=qkt[:, (2 + h) * S + kt * 128 : (2 + h) * S + (kt + 1) * 128],
                                    rhs=qkt[:, h * S + qb * QB + off : h * S + (qb + 1) * QB],
                                    start=True,
                                    stop=not diag,
                                )
                                if diag:
                                    # additive triangle on the first 128 valid
                                    # cols via identity-weight matmul
                                    nc.tensor.matmul(
                                        psS[:, 0:128],
                                        lhsT=masks_sb[:, 128:256],
                                        rhs=masks_sb[:, 0:128],
                                        start=False,
                                        stop=True,
                                    )
                                # exp lands q-aligned in p; pad cols are
                                # never read (attn@V / denom are narrowed)
                                nc.scalar.activation(
                                    p[:, u * QB + off : (u + 1) * QB],
                                    psS[:, 0:w],
                                    mybir.ActivationFunctionType.Exp,
                                    scale=float(SCALE),
                                )
                            ps_info[h] = (p, offs)
                        # attn@V + denominators, narrowed to the causally
                        # valid q-columns on diagonal k-tiles
                        for h in range(NH):
                            p, offs = ps_info[h]
                            for u in range(2):
                                kt = kt0 + u
                                off = offs[u]
                                nc.tensor.matmul(
                                    psO[h][:, off:QB],
                                    lhsT=vt[:, kt * DL + h * 128 : kt * DL + (h + 1) * 128],
                                    rhs=p[:, u * QB + off : (u + 1) * QB],
                                    start=(kt == 0),
                                    stop=(kt == n_k - 1),
                                )
                            # denominator: DVE pre-sums the two k-tiles (bf16,
                            # off the critical path), then ONE ones-matmul per
                            # pair — halves the PE denominator column count
                            off0, off1 = offs
                            psum2 = pp.tile([128, QB], BF16, name="psum2", tag="ps2", bufs=4)
                            if off1 > off0:
                                # u0's exclusive strip, then the common range
                                nc.vector.tensor_copy(
                                    psum2[:, off0:off1], p[:, off0:off1]
                                )
                                nc.vector.tensor_add(
                                    psum2[:, off1:QB],
                                    p[:, off1:QB],
                                    p[:, QB + off1 : 2 * QB],
                                )
                            else:
                                nc.vector.tensor_add(
                                    psum2[:, :], p[:, 0:QB], p[:, QB : 2 * QB]
                                )
                            # second tree level: combine two pair-sums on DVE
                            # so PE streams one ones-matmul per FOUR k-tiles
                            if pi % 2 == 0:
                                pend[h] = (psum2, off0)
                            else:
                                prev, poff = pend.pop(h)
                                psum4 = pp.tile(
                                    [128, QB], BF16, name="psum4", tag="ps4", bufs=3
                                )
                                if off0 > poff:
                                    nc.vector.tensor_copy(
                                        psum4[:, poff:off0], prev[:, poff:off0]
                                    )
                                    nc.vector.tensor_add(
                                        psum4[:, off0:QB],
                                        prev[:, off0:QB],
                                        psum2[:, off0:QB],
                                    )
                                else:
                                    nc.vector.tensor_add(
                                        psum4[:, :], prev[:, :], psum2[:, :]
                                    )

                                # third tree level: psum4 is always full width
                                # (poff==0 — the diagonal only narrows inside
                                # psum2), so pair them too; PE streams one
                                # ones-matmul per EIGHT k-tiles
                                def emit_den(rhs_t, start, stop, h=h):
                                    nc.tensor.matmul(
                                        psD[32 * h : 32 * h + 1, :],
                                        lhsT=ones_col[:, :],
                                        rhs=rhs_t[:, :],
                                        start=start,
                                        stop=stop,
                                        tile_position=(0, 32 * h),
                                    )

                                if h in pend4:
                                    prev4 = pend4.pop(h)
                                    psum8 = pp.tile(
                                        [128, QB], BF16, name="psum8", tag="ps8", bufs=2
                                    )
                                    nc.vector.tensor_add(
                                        psum8[:, :], prev4[:, :], psum4[:, :]
                                    )
                                    emit_den(
                                        psum8, first_mm[h], pi == npairs - 1
                                    )
                                    first_mm[h] = False
                                elif pi == npairs - 1:
                                    # odd number of psum4s — emit the last one
                                    emit_den(psum4, first_mm[h], True)
                                    first_mm[h] = False
                                else:
                                    pend4[h] = psum4
                            if kt0 + 2 >= n_k:
                                emit_norm_pre(h)
                        if kt0 + 2 >= n_k:
                            for h in range(NH):
                                emit_norm_mul(h)
                        # deferred O-projection right before the next pair's
                        # scores — ready matmuls sit exactly where the psS
                        # WAR (exp completion) stall would otherwise land.
                        # Not during qb=1: the first item would wait on the
                        # DVE backlog (projection-phase V drains) ahead of
                        # qb0's normalize.
                        if qb >= 2:
                            # deeper pops early: qb2-3 still have exp-WAR
                            # stalls and a deep backlog of ready items
                            for _ in range(3 if qb <= 3 else 2):
                                if proj_items:
                                    emit_proj(*proj_items.pop(0))

                    # this block's O-projection items join the global queue;
                    # they drain at <=2 per pair over the REMAINING blocks
                    # (outt regions are never overwritten, so items can drift
                    # arbitrarily late)
                    proj_items.extend(
                        (qb, j, et) for j in range(4) for et in range(4)
                    )

                # tail: O-projection of the last q-block, alternating PSUM
                # banks so drains overlap the next pair of matmuls
                ti = 0
                while proj_items:
                    emit_proj(*proj_items.pop(0), alt=(ti % 2 == 1))
                    ti += 1
    nc.finalize()
    return nc


def _bf16(a: np.ndarray) -> np.ndarray:
    return np.ascontiguousarray(a.astype(ml_dtypes.bfloat16))


def make_in_maps(X, Wq, bq, Wk, bk, Wv, bv, Wo, is_causal: bool):
    x2d = np.asarray(X, dtype=np.float32).reshape(S, D)
    xt = _bf16(x2d.T)
    masks = np.zeros((2, 128, 128), dtype=ml_dtypes.bfloat16)
    if is_causal:
        ki = np.arange(128)[:, None]
        cj = np.arange(128)[None, :]
        masks[0] = np.where(ki <= cj, 0.0, -1e9).astype(ml_dtypes.bfloat16)
        masks[1] = np.eye(128, dtype=ml_dtypes.bfloat16)

    in_maps = []
    for c in range(NCORES):
        sl = slice(c * DL, (c + 1) * DL)
        in_maps.append(
            {
                "xt": xt,
                "wqt": _bf16(np.asarray(Wq)[sl, :].T),
                "wkt": _bf16(np.asarray(Wk)[sl, :].T),
                "wvt": _bf16(np.asarray(Wv)[sl, :].T),
                "bqkc": np.ascontiguousarray(
                    np.stack(
                        [
                            np.asarray(bq, dtype=np.float32)[sl][:128],
                            np.asarray(bq, dtype=np.float32)[sl][128:],
                            np.asarray(bk, dtype=np.float32)[sl][:128],
                            np.asarray(bk, dtype=np.float32)[sl][128:],
                        ],
                        axis=1,
                    )
                ),
                "bvrow": _bf16(np.asarray(bv)[None, sl]),
                "wot": _bf16(np.asarray(Wo)[:, sl].T),
                "masks": masks,
            }
        )
    return in_maps


_NC_CACHE: dict = {}


def _get_nc(is_causal: bool) -> bass.Bass:
    if is_causal not in _NC_CACHE:
        _NC_CACHE[is_causal] = build_nc(is_causal)
    return _NC_CACHE[is_causal]


def kernel(X, Wq, bq, Wk, bk, Wv, bv, Wo, bo, is_causal, **run_kwargs):
    causal = bool(int(np.asarray(is_causal)))
    nc = _get_nc(causal)
    in_maps = make_in_maps(X, Wq, bq, Wk, bk, Wv, bv, Wo, causal)
    res = run_bass_kernel_spmd(nc, in_maps, core_ids=list(range(NCORES)), **run_kwargs)
    out = np.asarray(bo, dtype=np.float32)[None, :].repeat(S, axis=0)
    for c in range(NCORES):
        out += np.asarray(res.results[c]["out"], dtype=np.float32)
    return out.reshape(1, S, D)
